# revision 6
# baseline (speedup 1.0000x reference)
"""Phi^4 lattice action on Trainium2 (Bass/Tile), 8-core data parallel. v4.

out[b] = LAM*sum phi^4 - S_x - S_y        (since 2 + 0.5*M_SQ == 0)

Host sends phi as BF16 in a halo layout (65 rows x 66 cols per state),
halving DMA bytes vs fp32. Tiles are loaded in PAIRS (one 2.1MB DMA).

Engine split (HW-measured: DVE STT+accum ~4.4us @1x, DVE TT ~2.2us @2x,
ACT activation ~3.7us @1x, all per 4096-elem tile pass):
  ACT: m1 = Square(phi) (pair-merged where it needs no accum), then
       Square(sqrt(LAM)*m1)+accum per tile -> LAM*sum phi^4.
  DVE: x-products via scalar_tensor_tensor with fused accum (1x).
       y-products: normally STT; on the REBAL pair (tiles 4,5) a
       pair-merged TT-add @2x computes b = phi + phi_sy and ACT squares
       it:  sum phi*phi_sy = 0.5*sum b^2 - sum phi^2   (sum phi^2 rides
       the m1 pass's accum_out), balancing DVE ~68us / ACT ~67us busy.
Tile 0 is DMA-chunked (first chunk 3 lattice rows) and processed in
quarters/halves so both engines start as early as the runtime allows; a
dummy Square preloads the ACT spline tables off the critical path.
Results for tiles 0-5 are reduced and stored early; only tiles 6,7
remain after the last compute pass.

Accumulating ops are never pair-merged: accum_out is a per-partition
scalar, and the two tiles of a pair hold different states on the same
partition.

Non-lattice shift inputs use a generic fp32 fallback path (host gathers
neighbour sums; device does fused multiply-reduce).
"""

import json
import math

import numpy as np
import ml_dtypes

import concourse.bass as bass
import concourse.mybir as mybir
import concourse.tile as tile
from concourse.bass_utils import run_bass_kernel_spmd

def _max_waits(opcode: str) -> int:
    # This walrus build accepts at most ONE sync wait per instruction.
    return 1


def _split_excess_waits(bir_bytes: bytes) -> bytes:
    """Peel excess sync waits onto injected same-engine Drain instructions
    (walrus rejects instructions carrying too many waits)."""
    bir = json.loads(bir_bytes)
    n_new = 0
    for func in bir.get("functions", []):
        for bb in func.get("blocks", []):
            insts = bb.get("instructions", [])
            out = []
            for inst in insts:
                sync = inst.get("sync_info") or {}
                waits = sync.get("on_wait") or []
                cap = _max_waits(inst["opcode"])
                if len(waits) > cap:
                    extra = waits[: len(waits) - cap]
                    keep = waits[len(waits) - cap :]
                    while extra:
                        chunk, extra = extra[:1], extra[1:]
                        out.append(
                            {
                                "debug": inst.get("debug", 0),
                                "engine": inst["engine"],
                                "ins": [],
                                "name": f"{inst['name']}-wsplit{n_new}",
                                "opcode": "Drain",
                                "outs": [],
                                "sync_info": {
                                    "on_update": [],
                                    "on_wait": chunk,
                                },
                            }
                        )
                        n_new += 1
                    sync["on_wait"] = keep
                    inst["sync_info"] = sync
                out.append(inst)
            bb["instructions"] = out
    return json.dumps(bir).encode()


def _patch_json(nc):
    orig = nc.to_json_bytes

    def patched():
        return _split_excess_waits(orig())

    nc.to_json_bytes = patched
    return nc

L = 64
N = L * L  # 4096
B = 8192
NCORES = 8
BPC = B // NCORES  # 1024 rows per core
P = 128
NTILES = BPC // P  # 8
NPAIRS = NTILES // 2  # 4

M_SQ = -4.0
LAM = 6.975
C2 = 2.0 + 0.5 * M_SQ  # == 0.0 for the reference constants
SQRT_LAM = math.sqrt(LAM)
INV_SQRT2 = 1.0 / math.sqrt(2.0)

TRACE = False
LAST_EXEC_NS = None

_f32 = mybir.dt.float32
_bf16 = mybir.dt.bfloat16


def _neighbours(length):
    idx = np.arange(length * length).reshape(length, length)
    shifts = [
        np.roll(idx, -1, axis=1),
        np.roll(idx, 1, axis=1),
        np.roll(idx, -1, axis=0),
        np.roll(idx, 1, axis=0),
    ]
    return np.stack([s.reshape(-1) for s in shifts], axis=0)


def _is_canonical_lattice(shift: np.ndarray) -> bool:
    if shift.shape != (4, N):
        return False
    exp = np.sort(_neighbours(L), axis=0)
    got = np.sort(shift.astype(np.int64), axis=0)
    return bool(np.array_equal(exp, got))


HR = L + 1   # 65 rows (row 64 = row 0)
WC = L + 2   # 66 cols (col 64 = col 0, col 65 pad) -- even row stride
NP = HR * WC  # 4290 padded elements per state

CPT = 12         # kacc columns per tile (tile 0 uses 10)
REBAL_PAIR = 2   # tiles 4,5: y-product via the square identity
# tile-0 DMA chunks (lattice-row boundaries; first chunk tiny for ramp)
RB = [0, 5, 22, 42, 64]
CH = [0, 6 * WC, 23 * WC, 43 * WC, NP]


def _build_lattice():
    nc = bass.Bass()
    mult = mybir.AluOpType.mult
    Square = mybir.ActivationFunctionType.Square

    phi = nc.dram_tensor("phi", [BPC, NP], _bf16, kind="ExternalInput")
    act = nc.dram_tensor("act", [P, NTILES * CPT + 4], _f32,
                         kind="ExternalOutput")

    with tile.TileContext(nc) as tc:
        with (
            tc.tile_pool(name="io", bufs=2) as io,
            tc.tile_pool(name="sq", bufs=2) as sqp,
            tc.tile_pool(name="bb", bufs=2) as bbp,
            tc.tile_pool(name="junk", bufs=1) as junkp,
            tc.tile_pool(name="accs", bufs=1) as accp,
            tc.tile_pool(name="resp", bufs=1) as resp,
        ):
            kacc = accp.tile([P, NTILES * CPT], _f32)
            nacc = accp.tile([P, 4], _f32)  # 0.5*sum b^2 (negative cols)
            nc.vector.memset(kacc, 0.0)
            nc.vector.memset(nacc, 0.0)
            kview = kacc.rearrange("p (t c) -> p t c", c=CPT)
            res = resp.tile([P, NTILES], _f32)
            warm = resp.tile([P, 1], _f32, tag="warm")
            # preload the ACT spline table set before any data arrives
            nc.scalar.square(warm, nacc[:, 0:1])

            for pr in range(NPAIRS):
                t0, t1 = 2 * pr, 2 * pr + 1
                x2 = io.tile([P, 2 * NP], _bf16)
                if pr == 0:
                    rows0 = phi[t0 * P : (t0 + 1) * P, :]
                    for k in range(4):
                        nc.sync.dma_start(
                            out=x2[:, CH[k] : CH[k + 1]],
                            in_=rows0[:, CH[k] : CH[k + 1]],
                        )
                    nc.sync.dma_start(
                        out=x2[:, NP : 2 * NP],
                        in_=phi[t1 * P : (t1 + 1) * P, :],
                    )
                else:
                    nc.sync.dma_start(
                        out=x2.rearrange("p (b e) -> p b e", b=2),
                        in_=phi[t0 * P : (t1 + 1) * P, :].rearrange(
                            "(b a) e -> a b e", b=2
                        ),
                    )
                xp = x2.rearrange("p (u r c) -> p u r c", u=2, c=WC)
                lat2 = xp[:, :, 0:L, 0:L]
                xsh2 = xp[:, :, 0:L, 1 : L + 1]
                ysh2 = xp[:, :, 1 : L + 1, 0:L]
                ca, cb = t0 * CPT, t1 * CPT
                rebal = pr == REBAL_PAIR

                m1 = sqp.tile([P, 2 * N], _bf16)
                m1v = m1.rearrange("p (u r c) -> p u r c", u=2, c=L)
                jact = junkp.tile([P, 2 * N], _bf16, tag="jact")
                jactv = jact.rearrange("p (u r c) -> p u r c", u=2, c=L)
                jx = junkp.tile([P, 2 * N], _bf16, tag="jx")
                jxv = jx.rearrange("p (u r c) -> p u r c", u=2, c=L)

                if pr == 0:
                    x3a, x3b = xp[:, 0], xp[:, 1]
                    # ACT tile 0 in chunk-matched quarters, tile 1 whole
                    for k in range(4):
                        r0, r1 = RB[k], RB[k + 1]
                        nc.scalar.square(
                            m1v[:, 0, r0:r1, :], x3a[:, r0:r1, 0:L]
                        )
                        nc.scalar.activation(
                            jactv[:, 0, r0:r1, :], m1v[:, 0, r0:r1, :],
                            Square, scale=SQRT_LAM,
                            accum_out=kacc[:, ca + 4 + k : ca + 5 + k],
                        )
                    nc.scalar.square(m1v[:, 1], x3b[:, 0:L, 0:L])
                    nc.scalar.activation(
                        jactv[:, 1], m1v[:, 1], Square, scale=SQRT_LAM,
                        accum_out=kacc[:, cb : cb + 1],
                    )
                    # DVE tile 0: x in quarters, y in halves
                    for k in range(4):
                        nc.vector.scalar_tensor_tensor(
                            out=jxv[:, 0, RB[k] : RB[k + 1], :],
                            in0=x3a[:, RB[k] : RB[k + 1], 1 : L + 1],
                            scalar=-1.0,
                            in1=x3a[:, RB[k] : RB[k + 1], 0:L],
                            op0=mult, op1=mult,
                            accum_out=kacc[:, ca + k : ca + 1 + k],
                        )
                    for h, (r0, r1) in enumerate([(0, 18), (18, 64)]):
                        nc.vector.scalar_tensor_tensor(
                            out=jxv[:, 1, r0:r1, :],
                            in0=x3a[:, r0 + 1 : r1 + 1, 0:L],
                            scalar=-1.0,
                            in1=x3a[:, r0:r1, 0:L],
                            op0=mult, op1=mult,
                            accum_out=kacc[:, ca + 8 + h : ca + 9 + h],
                        )
                    # tile 1: plain
                    nc.vector.scalar_tensor_tensor(
                        out=jxv[:, 0], in0=x3b[:, 0:L, 1 : L + 1],
                        scalar=-1.0, in1=x3b[:, 0:L, 0:L],
                        op0=mult, op1=mult,
                        accum_out=kacc[:, cb + 1 : cb + 2],
                    )
                    nc.vector.scalar_tensor_tensor(
                        out=jxv[:, 1], in0=x3b[:, 1 : L + 1, 0:L],
                        scalar=-1.0, in1=x3b[:, 0:L, 0:L],
                        op0=mult, op1=mult,
                        accum_out=kacc[:, cb + 2 : cb + 3],
                    )
                elif rebal:
                    # tile 4: full square-identity y; tile 5: only rows
                    # HB.. (the other half stays a plain STT) -> n = 1.5
                    # rebalanced units, matching DVE/ACT busy times.
                    HB = 40
                    b = bbp.tile([P, 2 * N], _bf16)
                    bv = b.rearrange("p (u r c) -> p u r c", u=2, c=L)
                    nc.vector.tensor_tensor(
                        out=bv[:, 0], in0=ysh2[:, 0], in1=lat2[:, 0],
                        op=mybir.AluOpType.add,
                    )
                    nc.vector.tensor_tensor(
                        out=bv[:, 1, HB:L, :],
                        in0=ysh2[:, 1, HB:L, :],
                        in1=lat2[:, 1, HB:L, :],
                        op=mybir.AluOpType.add,
                    )
                    # tile 4: m1 with full sum phi^2 accum
                    nc.scalar.activation(
                        m1v[:, 0], lat2[:, 0], Square,
                        accum_out=kacc[:, ca + 3 : ca + 4],
                    )
                    nc.scalar.activation(
                        jactv[:, 0], m1v[:, 0], Square, scale=SQRT_LAM,
                        accum_out=kacc[:, ca : ca + 1],
                    )
                    nc.scalar.activation(
                        jactv[:, 0], bv[:, 0], Square, scale=INV_SQRT2,
                        accum_out=nacc[:, 0:1],
                    )
                    nc.vector.scalar_tensor_tensor(
                        out=jxv[:, 0],
                        in0=xsh2[:, 0], scalar=-1.0, in1=lat2[:, 0],
                        op0=mult, op1=mult,
                        accum_out=kacc[:, ca + 1 : ca + 2],
                    )
                    # tile 5: m1 halves (accum sum phi^2 over HB.. only)
                    nc.scalar.activation(
                        m1v[:, 1, 0:HB, :], lat2[:, 1, 0:HB, :], Square,
                    )
                    nc.scalar.activation(
                        m1v[:, 1, HB:L, :], lat2[:, 1, HB:L, :], Square,
                        accum_out=kacc[:, cb + 3 : cb + 4],
                    )
                    nc.scalar.activation(
                        jactv[:, 1], m1v[:, 1], Square, scale=SQRT_LAM,
                        accum_out=kacc[:, cb : cb + 1],
                    )
                    nc.scalar.activation(
                        jactv[:, 1, HB:L, :], bv[:, 1, HB:L, :], Square,
                        scale=INV_SQRT2,
                        accum_out=nacc[:, 1:2],
                    )
                    nc.vector.scalar_tensor_tensor(
                        out=jxv[:, 1],
                        in0=xsh2[:, 1], scalar=-1.0, in1=lat2[:, 1],
                        op0=mult, op1=mult,
                        accum_out=kacc[:, cb + 1 : cb + 2],
                    )
                    nc.vector.scalar_tensor_tensor(
                        out=jxv[:, 0, 0:HB, :],
                        in0=ysh2[:, 1, 0:HB, :], scalar=-1.0,
                        in1=lat2[:, 1, 0:HB, :],
                        op0=mult, op1=mult,
                        accum_out=kacc[:, cb + 2 : cb + 3],
                    )
                else:
                    # pair-merged m1 (no accum); per-tile everything else
                    nc.scalar.activation(m1v, lat2, Square)
                    jy = junkp.tile([P, 2 * N], _bf16, tag="jy")
                    jyv = jy.rearrange("p (u r c) -> p u r c", u=2, c=L)
                    for u, cc in ((0, ca), (1, cb)):
                        nc.scalar.activation(
                            jactv[:, u], m1v[:, u], Square,
                            scale=SQRT_LAM,
                            accum_out=kacc[:, cc : cc + 1],
                        )
                        nc.vector.scalar_tensor_tensor(
                            out=jxv[:, u],
                            in0=xsh2[:, u], scalar=-1.0, in1=lat2[:, u],
                            op0=mult, op1=mult,
                            accum_out=kacc[:, cc + 1 : cc + 2],
                        )
                        nc.vector.scalar_tensor_tensor(
                            out=jyv[:, u],
                            in0=ysh2[:, u], scalar=-1.0, in1=lat2[:, u],
                            op0=mult, op1=mult,
                            accum_out=kacc[:, cc + 2 : cc + 3],
                        )

                if pr == REBAL_PAIR:  # tiles 0..5 done -> early store
                    nc.sync.dma_start(
                        out=act[:, 0 : 6 * CPT], in_=kacc[:, 0 : 6 * CPT]
                    )
                    nc.sync.dma_start(
                        out=act[:, NTILES * CPT :], in_=nacc
                    )

            nc.sync.dma_start(
                out=act[:, 6 * CPT : NTILES * CPT],
                in_=kacc[:, 6 * CPT :],
            )
    return nc


def _build_generic():
    """fp32 fallback: host precomputes nsum = sum_s phi[:, shift[s]]."""
    nc = bass.Bass()
    mult = mybir.AluOpType.mult
    Square = mybir.ActivationFunctionType.Square
    phi = nc.dram_tensor("phi", [BPC, N], _f32, kind="ExternalInput")
    nsum = nc.dram_tensor("nsum", [BPC, N], _f32, kind="ExternalInput")
    act = nc.dram_tensor("act", [P, NTILES], _f32, kind="ExternalOutput")
    with tile.TileContext(nc) as tc:
        with (
            tc.tile_pool(name="io", bufs=2) as io,
            tc.tile_pool(name="sq", bufs=2) as sqp,
            tc.tile_pool(name="junk", bufs=2) as junkp,
            tc.tile_pool(name="accs", bufs=1) as accp,
            tc.tile_pool(name="resp", bufs=1) as resp,
        ):
            kacc = accp.tile([P, NTILES * 2], _f32)
            nc.vector.memset(kacc, 0.0)
            res = resp.tile([P, NTILES], _f32)
            kview = kacc.rearrange("p (t c) -> p t c", c=2)
            for t in range(NTILES):
                x = io.tile([P, N], _f32, tag="x")
                ns = io.tile([P, N], _f32, tag="ns")
                nc.sync.dma_start(out=x, in_=phi[t * P : (t + 1) * P, :])
                nc.sync.dma_start(out=ns, in_=nsum[t * P : (t + 1) * P, :])
                a = sqp.tile([P, N], _f32)
                jact = junkp.tile([P, N], _bf16, tag="jact")
                nc.scalar.square(a, x)
                nc.scalar.activation(
                    jact, a, Square, scale=SQRT_LAM,
                    accum_out=kacc[:, 2 * t : 2 * t + 1],
                )
                jd = junkp.tile([P, N], _bf16, tag="jd")
                nc.vector.scalar_tensor_tensor(
                    out=jd, in0=ns, scalar=-0.5, in1=x,
                    op0=mult, op1=mult,
                    accum_out=kacc[:, 2 * t + 1 : 2 * t + 2],
                )
            nc.vector.reduce_sum(
                out=res, in_=kview, axis=mybir.AxisListType.X
            )
            nc.sync.dma_start(out=act[:, :], in_=res)
    assert C2 == 0.0
    return nc


_cache = {}


def _get(generic: bool):
    if generic not in _cache:
        _cache[generic] = _patch_json(
            _build_generic() if generic else _build_lattice()
        )
    return _cache[generic]


def kernel(phi_state, shift):
    global LAST_EXEC_NS
    phi = np.ascontiguousarray(np.asarray(phi_state, dtype=np.float32))
    assert phi.shape == (B, N), phi.shape
    shift_np = np.asarray(shift)

    if _is_canonical_lattice(shift_np):
        nc = _get(False)
        lat = phi.reshape(B, L, L).astype(ml_dtypes.bfloat16)
        xp = np.zeros((B, HR, WC), dtype=ml_dtypes.bfloat16)
        xp[:, 0:L, 0:L] = lat
        xp[:, 0:L, L] = lat[:, :, 0]     # x wrap column
        xp[:, L, 0:L] = lat[:, 0, :]     # y wrap row
        xp = xp.reshape(B, NP)
        in_maps = [
            {"phi": xp[i * BPC : (i + 1) * BPC]} for i in range(NCORES)
        ]
    else:
        nsum = np.zeros_like(phi)
        for s in range(shift_np.shape[0]):
            nsum += phi[:, shift_np[s].astype(np.int64)]
        nc = _get(True)
        in_maps = [
            {
                "phi": phi[i * BPC : (i + 1) * BPC],
                "nsum": nsum[i * BPC : (i + 1) * BPC],
            }
            for i in range(NCORES)
        ]

    r = run_bass_kernel_spmd(
        nc, in_maps, core_ids=list(range(NCORES)), trace=TRACE
    )
    LAST_EXEC_NS = r.exec_time_ns
    if _is_canonical_lattice(shift_np):
        outs = []
        for m in r.results:
            a = m["act"]  # [P, NTILES*CPT + 4]
            kc = a[:, : NTILES * CPT].reshape(P, NTILES, CPT)
            na = a[:, NTILES * CPT :]
            res = kc.sum(axis=2)  # [P, NTILES]
            res[:, 4] -= na[:, 0]
            res[:, 5] -= na[:, 1]
            outs.append(res.T.reshape(BPC, 1))
        out = np.concatenate(outs, axis=0)
    else:
        out = np.concatenate(
            [m["act"].T.reshape(BPC, 1) for m in r.results], axis=0
        )
    return out.astype(np.float32)
